# revision 51
# baseline (speedup 1.0000x reference)
"""DecoderRNN Trainium2 kernel: 63-step LSTM + Luong attention + vocab projection.

Strategy (8 NeuronCores, SPMD), v8 — fully replicated recurrence, fp16:
  - Every core runs the FULL LSTM recurrence locally (256 small matmuls per
    step stream at ~1 col/cycle with LDWEIGHTS hidden, so the full h @ W_hh
    costs only ~3.5us warm) -- NO per-step collective, no h exchange at all.
    The only cross-core traffic is a per-block background AllGather of the
    h-independent Xg = W_ih x_t + bias terms (computed TP-8, gathered one
    block ahead, fully hidden behind the recurrence).
  - Gates accumulate in PSUM on top of a DVE-preloaded Xg (no start flags
    after t=0; has_written bits persist so start=False matmuls accumulate).
  - Attention/decoder/vocab work for a finished block of steps is emitted as
    filler quanta between recurrence steps; each quantum's psum-consuming
    "finisher" (copies/activations/output DMAs) is deferred one quantum so
    it never head-of-line-blocks the recurrence's elementwise ops in the
    in-order ACT/DVE queues. Softmax uses exp(x)=(1+t)/(1-t), t=tanh(x/2)
    (max-shifted, numerically safe) keeping one ACT table set -- no reloads.
  - All-fp16 datapath (weights, h, enc, probs, dec, logits): full PE rate,
    ~4x lower quantization error than bf16 (everything is in fp16 range).
  - Block cols are b-major (b,t); vocab m-tiles are fixed 128-col windows
    (FWL) and output DMAs split at batch boundaries. Logits emitted fp16;
    host casts to f32 (and adds b_out if nonzero). Vocab V-sharded 8 ways.
"""

import numpy as np
import ml_dtypes
from contextlib import ExitStack

import concourse.bass as bass
import concourse.bacc as bacc
import concourse.tile as tile
import concourse.mybir as mybir
from concourse import masks
from concourse.bass_utils import run_bass_kernel_spmd

F32 = mybir.dt.float32
FP16 = mybir.dt.float16
AF = mybir.ActivationFunctionType
ALU = mybir.AluOpType
AX = mybir.AxisListType

B, T, S = 32, 63, 64          # batch, steps (T-1 of the 64), source len
V, E, H = 32000, 512, 1024
P = 128                       # partitions
NCORES = 8
R = T * B                     # 2016 rows, recurrence col index r = t*32 + b
VL = V // NCORES              # 4000 vocab cols per core
KH = H // P                   # 8 k-chunks over hidden
KE = E // P                   # 4 k-chunks over embedding
NQ = 4                        # gate quarters (i, f, o, g order on chip)
RING = 24                     # hall ring slots
VN = 8                        # vocab n-tiles of 500
VT = VL // VN                 # 500
BLOCK_SIZES = [12, 12, 12, 12, 8, 4, 3]
assert sum(BLOCK_SIZES) == T
BLOCKS = []
_t0 = 0
for _bs in BLOCK_SIZES:
    BLOCKS.append((_t0, _t0 + _bs))
    _t0 += _bs


def _dma_segments(m0, mw, bs):
    """Split dect col window [m0, m0+mw) at batch boundaries into (b, t)
    rectangles: (b_start, t_off, t_take, rel_row, n_batches)."""
    raw = []
    r = m0
    while r < m0 + mw:
        b, off = divmod(r, bs)
        take = min(m0 + mw - r, bs - off)
        raw.append((b, off, take, r - m0))
        r += take
    merged = []
    for b, off, take, rel in raw:
        if (merged and off == 0 and take == bs and merged[-1][1] == 0
                and merged[-1][2] == bs and merged[-1][0] + merged[-1][4] == b):
            merged[-1] = merged[-1][:4] + (merged[-1][4] + 1,)
            continue
        merged.append((b, off, take, rel, 1))
    return merged


def build_graph():
    nc = bacc.Bacc("TRN2", target_bir_lowering=False, debug=False,
                   num_devices=NCORES)

    def inp(name, shape, dtype):
        return nc.dram_tensor(name, list(shape), dtype, kind="ExternalInput").ap()

    x_embT = inp("x_embT", [E, R], FP16)            # embedded tgt, (k p) x (t,b)
    w_ihT_s = inp("w_ihT_s", [E, NQ * P], FP16)     # own TP slice, (c,p) cols
    w_hhT_f = inp("w_hhT_f", [H, 4 * H], FP16)      # FULL, cols (c, kout, p)
    bias_s = inp("bias_s", [P, NQ], F32)            # own chunk bias (c cols)
    h0T = inp("h0T", [H, B], FP16)
    c0T = inp("c0T", [H, B], F32)
    enc = inp("enc", [B, S, H], FP16)               # ctx lhsT
    encT = inp("encT", [B, H, S], FP16)             # scores rhs
    w_wT = inp("w_wT", [2 * H, H], FP16)
    b_w_sb = inp("b_w_sb", [P, KH], F32)
    w_outT_s = inp("w_outT_s", [H, VL], FP16)       # per-core vocab slice
    b_out_s = inp("b_out_s", [1, VL], FP16)
    out_s = nc.dram_tensor("out_s", [B, T, VL], FP16, kind="ExternalOutput").ap()

    x_embT_r = x_embT.rearrange("(k p) r -> p k r", p=P)

    with tile.TileContext(nc) as tc, ExitStack() as ctx:
        pool1 = ctx.enter_context(tc.tile_pool(name="pool1", bufs=1))
        stream = ctx.enter_context(tc.tile_pool(name="stream", bufs=2))
        work = ctx.enter_context(tc.tile_pool(name="work", bufs=2))
        state = ctx.enter_context(tc.tile_pool(name="state", bufs=2))
        psum = ctx.enter_context(tc.tile_pool(name="psum", bufs=2, space="PSUM"))
        dram = ctx.enter_context(tc.tile_pool(name="dram", bufs=1, space="DRAM"))

        # ---------------- resident tiles (small/critical first) -------------
        wih = pool1.tile([P, KE, NQ * P], FP16, name="wih")
        nc.sync.dma_start(wih[:], w_ihT_s.rearrange("(k p) c -> p k c", p=P))
        bias_t = pool1.tile([P, NQ], F32, name="bias_t")
        nc.sync.dma_start(bias_t[:], bias_s[:])
        bw_t = pool1.tile([P, KH], F32, name="bw_t")
        nc.sync.dma_start(bw_t[:], b_w_sb[:])
        h0_t = pool1.tile([P, KH, B], FP16, name="h0_t")
        nc.sync.dma_start(h0_t[:], h0T.rearrange("(k p) b -> p k b", p=P))
        c0_sb = pool1.tile([P, KH, B], F32, name="c0_sb")
        nc.sync.dma_start(c0_sb[:], c0T.rearrange("(k p) b -> p k b", p=P))
        ident = pool1.tile([P, P], FP16, name="ident")
        masks.make_identity(nc, ident[:])
        whh = pool1.tile([P, KH, 4 * H], FP16, name="whh")
        nc.sync.dma_start(whh[:], w_hhT_f.rearrange("(k p) c -> p k c", p=P))
        # big vocab weight on the scalar queue, needed later
        wout = pool1.tile([P, KH, VL], FP16, name="wout")
        nc.scalar.dma_start(wout[:], w_outT_s.rearrange("(k p) v -> p k v", p=P))

        # hall ring: full h (fp16) for the last RING steps
        hall = pool1.tile([P, KH, RING, B], FP16, name="hall")

        # per-block Xg pipeline buffers
        xg_in = [dram.tile([NQ * P, B * bs], FP16, name=f"xg_in{i}")
                 for i, bs in enumerate(BLOCK_SIZES)]
        xg_out = [dram.tile([NCORES * NQ * P, B * bs], FP16, name=f"xg_out{i}",
                            addr_space="Shared") for i, bs in
                  enumerate(BLOCK_SIZES)]

        xe_tiles = {}

        def fetch_xe(bi):
            t0, t1 = BLOCKS[bi]
            xe = stream.tile([P, KE, B * (t1 - t0)], FP16, name="xe",
                             tag=f"xe{bi % 2}", bufs=1)
            nc.sync.dma_start(xe[:], x_embT_r[:, :, t0 * B:t1 * B])
            xe_tiles[bi] = xe

        # Xg = W_ih x + bias for a block: computed TP-8 (own 4 gate chunks),
        # AllGathered in the background one block ahead of use.
        def emit_xg(bi):
            t0, t1 = BLOCKS[bi]
            cols = B * (t1 - t0)
            xe = xe_tiles[bi]
            xgs = work.tile([P, NQ, 384], FP16, name="xgs", tag="xgs", bufs=1)
            for c in range(NQ):
                ps_x = psum.tile([P, 512], F32, name="ps_x", tag="ps_big",
                                 bufs=1)
                for k in range(KE):
                    nc.tensor.matmul(
                        ps_x[:, :cols],
                        lhsT=wih[:, k, c * P:(c + 1) * P],
                        rhs=xe[:, k, :],
                        start=(k == 0), stop=(k == KE - 1))
                nc.scalar.activation(xgs[:, c, :cols], ps_x[:, :cols],
                                     AF.Identity, bias=bias_t[:, c:c + 1])
            nc.gpsimd.dma_start(
                xg_in[bi].rearrange("(c p) x -> p c x", p=P),
                xgs[:, :, :cols])
            nc.gpsimd.collective_compute(
                "AllGather", ALU.bypass,
                replica_groups=[list(range(NCORES))],
                ins=[xg_in[bi].opt()],
                outs=[xg_out[bi].opt()])

        # ---------------- filler emission (attention/dec/vocab per block) ----
        # Each closure emits matmuls and returns a "finisher" (psum-consuming
        # copies/activations/DMAs) that the drain runs one quantum later, so
        # a finisher never head-of-line-blocks the recurrence's elementwise
        # ops in the in-order ACT/DVE queues.
        def block_closures(bi):
            t0, t1 = BLOCKS[bi]
            bs = t1 - t0
            cols = B * bs            # block cols, b-major (b, t)
            r0 = t0 % RING
            cls = []

            pn_t = work.tile([16, B, S], FP16, name="pn", tag="pn_blk", bufs=1)
            at_t = work.tile([S, B, 16], FP16, name="at", tag="at_blk", bufs=1)
            ctxb = work.tile([P, KH, cols], FP16, name="ctxb", tag="ctx_blk",
                             bufs=1)
            decb = work.tile([P, KH, cols], FP16, name="decb", tag="dec_blk",
                             bufs=1)

            def mk_scores(q):
                def emit():
                    ps_s = psum.tile([16, 4, S], F32, name="ps_s", tag="ps_sc",
                                     bufs=2)
                    et4 = stream.tile([P, 4, KH, S], FP16, name="et4",
                                      tag="et4", bufs=1)
                    nc.sync.dma_start(
                        et4[:], encT[4 * q:4 * q + 4, :, :].rearrange(
                            "b (k p) s -> p b k s", p=P))
                    for bq in range(4):
                        b = q * 4 + bq
                        for k in range(KH):
                            nc.tensor.matmul(
                                ps_s[:bs, bq, :],
                                lhsT=hall[:, k, r0:r0 + bs, b],
                                rhs=et4[:, bq, k, :],
                                start=(k == 0), stop=(k == KH - 1))

                    def fin():
                        # softmax over s: exp(x) = (1+t)/(1-t), t = tanh(x/2)
                        mxn = work.tile([16, 1], F32, name="mxn", tag="mxn")
                        nc.vector.tensor_reduce(mxn[:bs, :], ps_s[:bs, :, :],
                                                axis=AX.XY, op=ALU.max,
                                                negate=True)
                        nmx2 = work.tile([16, 1], F32, name="nmx2", tag="nmx2")
                        nc.vector.tensor_scalar_mul(nmx2[:bs, :], mxn[:bs, :],
                                                    0.5)
                        tq = work.tile([16, 4, S], F32, name="tq", tag="tq",
                                       bufs=1)
                        nc.scalar.activation(tq[:bs, :, :], ps_s[:bs, :, :],
                                             AF.Tanh, bias=nmx2[:bs, :],
                                             scale=0.5)
                        un = work.tile([16, 4, S], F32, name="un", tag="un",
                                       bufs=1)
                        nc.vector.tensor_scalar_add(un[:bs, :, :],
                                                    tq[:bs, :, :], 1.0)
                        dn = work.tile([16, 4, S], F32, name="dn", tag="dn",
                                       bufs=1)
                        nc.vector.tensor_scalar(dn[:bs, :, :], tq[:bs, :, :],
                                                -1.0, 1.0, ALU.mult, ALU.add)
                        nc.vector.reciprocal(dn[:bs, :, :], dn[:bs, :, :])
                        pu = un
                        nc.vector.tensor_tensor(out=pu[:bs, :, :],
                                                in0=un[:bs, :, :],
                                                in1=dn[:bs, :, :], op=ALU.mult)
                        zs = work.tile([16, 4], F32, name="zs", tag="zs")
                        nc.vector.tensor_reduce(zs[:bs, :], pu[:bs, :, :],
                                                axis=AX.X, op=ALU.add)
                        rz = work.tile([16, 4], F32, name="rz", tag="rz")
                        nc.vector.reciprocal(rz[:bs, :], zs[:bs, :])
                        for bq in range(4):
                            b = q * 4 + bq
                            nc.vector.tensor_scalar_mul(
                                pn_t[:bs, b, :], pu[:bs, bq, :],
                                rz[:bs, bq:bq + 1])
                    return fin
                return emit

            def mk_transp(pg):
                def emit():
                    ps_t4 = psum.tile([S, 4, 16], FP16, name="ps_t4",
                                      tag="ps_tr", bufs=1)
                    for i, b in enumerate(range(4 * pg, 4 * pg + 4)):
                        nc.tensor.transpose(
                            ps_t4[:, i, :bs], pn_t[:bs, b, :], ident[:bs, :bs])

                    def fin():
                        for i, b in enumerate(range(4 * pg, 4 * pg + 4)):
                            nc.vector.tensor_copy(at_t[:, b, :bs],
                                                  ps_t4[:, i, :bs])
                    return fin
                return emit

            def mk_ctx(k):
                def emit():
                    eca = stream.tile([S, B, P], FP16, name="eca", tag="eca",
                                      bufs=1)
                    nc.sync.dma_start(
                        eca[:], enc[:, :, k * P:(k + 1) * P].rearrange(
                            "b s h -> s b h"))
                    ps_c = psum.tile([P, 512], F32, name="ps_c",
                                     tag="ps_d512", bufs=2)
                    for b in range(B):
                        nc.tensor.matmul(
                            ps_c[:, b * bs:(b + 1) * bs],
                            lhsT=eca[:, b, :],
                            rhs=at_t[:, b, :bs],
                            start=True, stop=True)

                    def fin():
                        nc.vector.tensor_copy(ctxb[:, k, :], ps_c[:, :cols])
                    return fin
                return emit

            def mk_dec(mo):
                def emit():
                    wws = stream.tile([P, 2 * KH, P], FP16, name="wws",
                                      tag="wws", bufs=2)
                    nc.sync.dma_start(
                        wws[:], w_wT.rearrange("(k p) m -> p k m", p=P)[
                            :, :, mo * P:(mo + 1) * P])
                    ps_d = psum.tile([P, 512], F32, name="ps_d", tag="ps_big",
                                     bufs=1)
                    for k in range(KH):
                        nc.tensor.matmul(
                            ps_d[:, :cols],
                            lhsT=wws[:, k, :],
                            rhs=hall[:, k, r0:r0 + bs, :].rearrange(
                                "p t b -> p b t"),
                            start=(k == 0), stop=False)
                    for k in range(KH):
                        nc.tensor.matmul(
                            ps_d[:, :cols],
                            lhsT=wws[:, KH + k, :],
                            rhs=ctxb[:, k, :],
                            start=False, stop=(k == KH - 1))

                    def fin():
                        nc.scalar.activation(decb[:, mo, :], ps_d[:, :cols],
                                             AF.Tanh, bias=bw_t[:, mo:mo + 1])
                    return fin
                return emit

            def mk_vocab(n, g):
                def emit():
                    m0 = g * P
                    mw = min(P, cols - m0)
                    ps_v = psum.tile([P, 512], F32, name="ps_v",
                                     tag="ps_d512", bufs=2)
                    for k in range(KH):
                        nc.tensor.matmul(
                            ps_v[:mw, :VT],
                            lhsT=decb[:, k, m0:m0 + mw],
                            rhs=wout[:, k, n * VT:(n + 1) * VT],
                            start=(k == 0), stop=(k == KH - 1))

                    def fin():
                        o_sb = work.tile([P, VT], FP16, name="o_sb",
                                         tag="o_sb", bufs=1)
                        nc.vector.tensor_copy(o_sb[:mw, :], ps_v[:mw, :VT])
                        for (b0, toff, ttake, rel, nb) in _dma_segments(
                                m0, mw, bs):
                            nc.scalar.dma_start(
                                out_s[b0:b0 + nb, t0 + toff:t0 + toff + ttake,
                                      n * VT:(n + 1) * VT],
                                o_sb[rel:rel + nb * ttake, :])
                    return fin
                return emit

            for q in range(8):
                cls.append(mk_scores(q))
            for pg in range(8):
                cls.append(mk_transp(pg))
            for k in range(KH):
                cls.append(mk_ctx(k))
            for mo in range(KH):
                cls.append(mk_dec(mo))
            for n in range(VN):
                for g in range(-(-cols // P)):
                    cls.append(mk_vocab(n, g))
            return cls

        # ---------------- recurrence with interleaved filler ----------------
        fetch_xe(0)
        emit_xg(0)
        fetch_xe(1)
        emit_xg(1)
        pending = []
        fin_q = []

        def drain_one():
            if fin_q:
                fin_q.pop(0)()
            if pending:
                f = pending.pop(0)()
                if f is not None:
                    fin_q.append(f)

        blocks_done = 0
        cur_blk = 0
        c_prev = c0_sb
        for t in range(T):
            if t >= BLOCKS[cur_blk][1]:
                cur_blk += 1
                if cur_blk + 1 < len(BLOCKS):
                    fetch_xe(cur_blk + 1)
                    emit_xg(cur_blk + 1)
            t0b = BLOCKS[cur_blk][0]
            rt = t % RING
            # stage this step's Xg [4096, 32] from the gathered block buffer
            xg_sb = stream.tile([P, KH, NQ, B], FP16, name="xg_sb",
                                tag="xg_sb", bufs=2)
            nc.sync.dma_start(
                xg_sb[:],
                xg_out[cur_blk].rearrange(
                    "(m c p) x -> p m c x", p=P, c=NQ)[
                    :, :, :, (t - t0b) * B:(t - t0b + 1) * B])
            # gates psum [p, c, kout, b] preloaded with Xg via DVE, then 256
            # W_hh matmuls accumulate (has_written persists; no start flags
            # after t=0, when two start=True identity matmuls seed the banks).
            psg = psum.tile([P, KH, NQ, B], F32, name="psg", tag="psg", bufs=1)
            if t == 0:
                for half in range(2):
                    nc.tensor.matmul(
                        psg[:, 4 * half:4 * half + 4, :, :].rearrange(
                            "p k c b -> p (k c b)"),
                        lhsT=ident[:],
                        rhs=xg_sb[:, 4 * half:4 * half + 4, :, :].rearrange(
                            "p k c b -> p (k c b)"),
                        start=True, stop=False, skip_group_check=True)
            else:
                nc.vector.tensor_copy(psg[:], xg_sb[:])
            for ki in range(KH):
                rhs = (h0_t[:, ki, :] if t == 0
                       else hall[:, ki, (t - 1) % RING, :])
                for ko in range(KH):
                    for c in range(NQ):
                        nc.tensor.matmul(
                            psg[:, ko, c, :],
                            lhsT=whh[:, ki, (ko * NQ + c) * P:
                                     (ko * NQ + c + 1) * P],
                            rhs=rhs,
                            start=False, stop=False, skip_group_check=True)
            # LSTM elementwise, full width, gates read straight from PSUM
            sfo = work.tile([P, KH, 3, B], F32, name="sfo", tag="sfo", bufs=1)
            nc.scalar.activation(sfo[:], psg[:, :, 0:3, :], AF.Sigmoid)
            tg = work.tile([P, KH, B], F32, name="tg", tag="tg", bufs=1)
            nc.scalar.activation(tg[:], psg[:, :, 3, :], AF.Tanh)
            t1_ = work.tile([P, KH, B], F32, name="t1_", tag="t1_", bufs=1)
            nc.vector.tensor_tensor(out=t1_[:], in0=sfo[:, :, 1, :],
                                    in1=c_prev[:], op=ALU.mult)
            t2_ = work.tile([P, KH, B], F32, name="t2_", tag="t2_", bufs=1)
            nc.vector.tensor_tensor(out=t2_[:], in0=sfo[:, :, 0, :],
                                    in1=tg[:], op=ALU.mult)
            c_new = state.tile([P, KH, B], F32, name="c_new", tag="c_new")
            nc.vector.tensor_tensor(out=c_new[:], in0=t1_[:], in1=t2_[:],
                                    op=ALU.add)
            c_prev = c_new
            tc_t = work.tile([P, KH, B], F32, name="tc_t", tag="tc_t", bufs=1)
            nc.scalar.activation(tc_t[:], c_new[:], AF.Tanh)
            nc.vector.tensor_tensor(out=hall[:, :, rt, :],
                                    in0=sfo[:, :, 2, :],
                                    in1=tc_t[:], op=ALU.mult)
            # drain filler
            if pending or fin_q:
                nxt = BLOCKS[blocks_done][1] if blocks_done < len(BLOCKS) else T
                quota = -(-len(pending) // max(1, nxt - t))
                for _ in range(max(quota, 1)):
                    drain_one()
            if blocks_done < len(BLOCKS) and t + 1 == BLOCKS[blocks_done][1]:
                pending.extend(block_closures(blocks_done))
                blocks_done += 1
        while pending or fin_q:
            drain_one()
    nc.compile()
    return nc


_CACHE = {}


def _get_graph():
    if "nc" not in _CACHE:
        _CACHE["nc"] = build_graph()
    return _CACHE["nc"]


QMAP = (0, 1, 3, 2)   # chunk order (i, f, o, g) -> PyTorch quarter index


def _prep(tgt_input, hidden_state, cell_state, encoder_outputs,
          embedding, W_ih, W_hh, b_ih, b_hh, W_w, b_w, W_out, b_out):
    """Host-side layout prep. Returns per-core input maps."""
    f32 = np.float32
    fp16 = np.float16
    idx = np.asarray(tgt_input)[:, :-1].astype(np.int64)    # [B, T]
    emb = np.asarray(embedding, f32)[idx]                   # [B, T, E]
    x_embT = np.ascontiguousarray(
        emb.transpose(2, 1, 0).reshape(E, R)).astype(fp16)

    w_ihT = np.asarray(W_ih, f32).T                         # [E, 4H]
    w_hhT = np.asarray(W_hh, f32).T                         # [H, 4H]
    bias = (np.asarray(b_ih, f32) + np.asarray(b_hh, f32))  # [4H]
    h0T_a = np.ascontiguousarray(
        np.asarray(hidden_state, f32)[0].T).astype(fp16)    # [H, B]
    c0T_a = np.ascontiguousarray(np.asarray(cell_state, f32)[0].T)  # [H, B]
    enc_b = np.asarray(encoder_outputs, f32).astype(fp16)   # [B, S, H]
    encT_b = np.ascontiguousarray(
        np.asarray(encoder_outputs, f32).transpose(0, 2, 1)).astype(fp16)
    w_wT_full = np.ascontiguousarray(np.asarray(W_w, f32).T).astype(fp16)
    b_w_sb = np.ascontiguousarray(np.asarray(b_w, f32).reshape(KH, P).T)
    w_outT = np.asarray(W_out, f32).T                       # [H, V]
    b_out_a = np.asarray(b_out, f32)

    # full W_hh with columns in device chunk order (c, kout)
    cols_full = np.concatenate(
        [np.arange(QMAP[c] * H + ko * P, QMAP[c] * H + (ko + 1) * P)
         for ko in range(KH) for c in range(NQ)])
    w_hhT_dev = np.ascontiguousarray(w_hhT[:, cols_full]).astype(fp16)

    in_maps = []
    for m in range(NCORES):
        cols = np.concatenate(
            [np.arange(QMAP[c] * H + m * P, QMAP[c] * H + (m + 1) * P)
             for c in range(NQ)])
        in_maps.append({
            "x_embT": x_embT,
            "w_ihT_s": np.ascontiguousarray(w_ihT[:, cols]).astype(fp16),
            "w_hhT_f": w_hhT_dev,
            "bias_s": np.ascontiguousarray(bias[cols].reshape(NQ, P).T),
            "h0T": h0T_a,
            "c0T": c0T_a,
            "enc": enc_b,
            "encT": encT_b,
            "w_wT": w_wT_full,
            "b_w_sb": b_w_sb,
            "w_outT_s": np.ascontiguousarray(
                w_outT[:, m * VL:(m + 1) * VL]).astype(fp16),
            "b_out_s": np.ascontiguousarray(
                b_out_a[m * VL:(m + 1) * VL]).reshape(1, VL).astype(fp16),
        })
    return in_maps


def kernel(**inputs) -> np.ndarray:
    nc = _get_graph()
    in_maps = _prep(**inputs)
    res = run_bass_kernel_spmd(nc, in_maps, list(range(NCORES)))
    outs = [np.asarray(res.results[m]["out_s"], dtype=np.float32)
            for m in range(NCORES)]
    full = np.concatenate(outs, axis=2)
    b_out = np.asarray(inputs["b_out"], np.float32)
    if np.any(b_out):
        full = full + b_out
    return full
